# revision 1
# baseline (speedup 1.0000x reference)
"""CapsuleLayer kernel, raw-Bass version (no Tile framework overhead).

Math: the reference's routing logits start at zero and the agreement
update is constant over the output-capsule axis, so softmax stays
uniform through all 3 routing iterations and the exact output is
  out[b, j, :] = squash(mean_n(x[b,n,:] @ W[0,n]))  for every j.

Per core (8 batch rows, data-parallel over B):
  - inputs xt [128, 72, 8] (x^T, contraction on partitions) and
    wf [128, 72, 16] (W pre-scaled by 1/N on host), each DMA'd in 4
    chunks (sync ring: xt, scalar ring: wf) so matmuls start as soon
    as the first quarter lands
  - 72 accumulating PE matmuls -> pm[8,16] = m  (fp32 or bf16 inputs)
  - squash on DVE/ACT with explicit same-engine completion sems (the
    DVE pipeline overlaps instructions; back-to-back dependent ops
    read stale data without them -- HW-verified)
  - broadcast across partitions via PE: vdiag[8,128] = block-diagonal
    (one DVE multiply against a memset-built 0/1 mask), then ONE
    matmul with a memset-built all-ones [8,128] stationary:
    pbc[p, b*16+d] = v[b, d] for all p
  - vb = copy(pbc) to SBUF; output o[128, 9, 8, 16] written by nine
    per-j-tile DMAs (3 per ring: sync / scalar / gpsimd-SWDGE), each a
    plain [128, 128] -> [128, 1, 128] transfer with real strides
"""

import os

import numpy as np

import concourse.bass as bass
import concourse.mybir as mybir
from concourse.bass_utils import run_bass_kernel_spmd

B, N, IN_DIM, OUT_DIM = 64, 1152, 8, 16
NCORES = 8
BPC = B // NCORES
K = N * IN_DIM
CK = K // 128          # 72 contraction chunks
TJ = N // 128          # 9 j-tiles
NQ = 4                 # input DMA quarters
CQ = CK // NQ          # 18 chunks per quarter
F32 = mybir.dt.float32
AF = mybir.ActivationFunctionType

USE_BF16 = os.environ.get("KERNEL_BF16", "0") == "1"

_CACHE = {}
LAST_RESULT = None


def build_nc(safe_sync=False, use_bf16=USE_BF16):
    in_dt = mybir.dt.bfloat16 if use_bf16 else F32
    nc = bass.Bass("TRN2", target_bir_lowering=False, debug=False)

    xt = nc.dram_tensor("xt", [128, CK, BPC], in_dt, kind="ExternalInput").ap()
    wf = nc.dram_tensor("wf", [128, CK, OUT_DIM], in_dt, kind="ExternalInput").ap()
    o = nc.dram_tensor("o", [128, TJ, BPC, OUT_DIM], F32, kind="ExternalOutput").ap()
    debug = os.environ.get("KERNEL_DEBUG") == "1"
    if debug:
        dbg_m = nc.dram_tensor("dbg_m", [BPC, OUT_DIM], F32, kind="ExternalOutput").ap()
        dbg_v = nc.dram_tensor("dbg_v", [BPC, OUT_DIM], F32, kind="ExternalOutput").ap()

    one = nc.const_aps.aps[(F32, 1.0)]

    from contextlib import ExitStack

    with ExitStack() as ctx:
        e = ctx.enter_context
        xt_t = e(nc.sbuf_tensor([128, CK * BPC], in_dt))
        wf_t = e(nc.sbuf_tensor([128, CK * OUT_DIM], in_dt))
        pm = e(nc.psum_tensor([BPC, OUT_DIM], F32))
        pbc = e(nc.psum_tensor([128, BPC * OUT_DIM], F32))
        m_s = e(nc.sbuf_tensor([BPC, OUT_DIM], F32))
        msq = e(nc.sbuf_tensor([BPC, OUT_DIM], F32))
        sq = e(nc.sbuf_tensor([BPC, 1], F32))
        s1 = e(nc.sbuf_tensor([BPC, 1], F32))
        den = e(nc.sbuf_tensor([BPC, 1], F32))
        rcp = e(nc.sbuf_tensor([BPC, 1], F32))
        v = e(nc.sbuf_tensor([BPC, OUT_DIM], F32))
        ones8 = e(nc.sbuf_tensor([BPC, 128], F32))
        dg01 = e(nc.sbuf_tensor([BPC, BPC * OUT_DIM], F32))
        vdiag = e(nc.sbuf_tensor([BPC, BPC * OUT_DIM], F32))
        vb = e(nc.sbuf_tensor([128, 4 * BPC * OUT_DIM], F32))
        eps_t = e(nc.sbuf_tensor([128, 1], F32))
        warm = e(nc.sbuf_tensor([1, 1], F32))
        sem_xq = [e(nc.semaphore(f"sem_x{q}")) for q in range(NQ)]
        sem_wq = [e(nc.semaphore(f"sem_w{q}")) for q in range(NQ)]
        sem_const = e(nc.semaphore("sem_const"))
        sem_mm = e(nc.semaphore("sem_mm"))
        sem_sq = e(nc.semaphore("sem_sq"))
        sem_s1 = e(nc.semaphore("sem_s1"))
        sem_v = e(nc.semaphore("sem_v"))
        sem_vd = e(nc.semaphore("sem_vd"))
        sem_bc = e(nc.semaphore("sem_bc"))
        sem_ob = e(nc.semaphore("sem_ob"))
        sem_o = e(nc.semaphore("sem_o"))
        sem_og = e(nc.semaphore("sem_og"))
        vsem = e(nc.semaphore("vsem"))
        msem = e(nc.semaphore("msem")) if safe_sync else None
        block = e(nc.Block(no_gpsimd_drain=True))

        vcount = [0]
        mcount = [0]

        def vchain(eng, instr):
            # same-engine RAW chaining: the DVE pipeline overlaps
            # instructions, dependent back-to-back ops need this.
            vcount[0] += 1
            instr.then_inc(vsem, 1)
            eng.wait_ge(vsem, vcount[0])
            return instr

        xt_v = xt_t.ap().rearrange("p (c b) -> p c b", b=BPC)
        wf_v = wf_t.ap().rearrange("p (c d) -> p c d", d=OUT_DIM)

        # out DMAs: one slab per ring (4 j-tiles on each HWDGE ring, 1 on
        # the slower gpsimd SWDGE ring), plain strided reads from vb
        def out_dma(eng, t0, t1, sem):
            eng.wait_ge(sem_ob, 1)
            eng.dma_start(
                out=o[:, t0:t1, :, :],
                in_=vb.ap().rearrange("p (t f) -> p t f", t=4)[:, : t1 - t0, :],
            ).then_inc(sem, 16)
            eng.wait_ge(sem_o, 16 * 2)
            eng.wait_ge(sem_og, 16)

        @block.sync
        def _(sync):
            for q in range(NQ):
                cs = slice(q * CQ, (q + 1) * CQ)
                sync.dma_start(out=xt_v[:, cs, :], in_=xt[:, cs, :]).then_inc(
                    sem_xq[q], 16
                )
            if debug:
                sync.wait_ge(sem_v, 1)
                sync.dma_start(out=dbg_m[:, :], in_=m_s[:, :]).then_inc(sem_o, 16)
                sync.dma_start(out=dbg_v[:, :], in_=v[:, :]).then_inc(sem_o, 16)
            out_dma(sync, 0, 4, sem_o)

        @block.scalar
        def _(scalar):
            for q in range(NQ):
                cs = slice(q * CQ, (q + 1) * CQ)
                scalar.dma_start(out=wf_v[:, cs, :], in_=wf[:, cs, :]).then_inc(
                    sem_wq[q], 16
                )
            # warm the Sqrt activation table while DMAs/matmuls run
            nc.scalar.activation(warm[:, :], one[:1, :], AF.Sqrt)
            scalar.wait_ge(sem_const, 1)   # eps_t is memset first
            scalar.wait_ge(sem_sq, 1)
            nc.scalar.activation(
                s1[:, :], sq[:, :], AF.Sqrt, bias=eps_t.ap()[:BPC, :]
            ).then_inc(sem_s1, 1)
            out_dma(scalar, 4, 8, sem_o)

        @block.gpsimd
        def _(gpsimd):
            # constants: eps (first), all-ones [8,128], 0/1 block-diagonal
            gpsimd.memset(eps_t.ap(), 1e-8).then_inc(sem_const, 1)
            gpsimd.memset(ones8.ap(), 1.0).then_inc(sem_const, 1)
            gpsimd.memset(dg01.ap(), 0.0).then_inc(sem_const, 1)
            gpsimd.wait_ge(sem_const, 3)
            # dg01[i, b*16+d] = (i == b) ? 1 : 0
            gpsimd.affine_select(
                out=dg01.ap().rearrange("i (b d) -> i b d", d=OUT_DIM),
                in_=dg01.ap().rearrange("i (b d) -> i b d", d=OUT_DIM),
                compare_op=mybir.AluOpType.not_equal,
                fill=1.0,
                base=0,
                pattern=[[-1, BPC], [0, OUT_DIM]],
                channel_multiplier=1,
            ).then_inc(sem_const, 1)
            out_dma(gpsimd, 8, 9, sem_og)

        @block.vector
        def _(vector):
            # squash: m = pm; sq = sum(m^2); s1 = sqrt(sq + 1e-8);
            # den = s1*(1+sq); v = m * sq / den
            vector.wait_ge(sem_mm, 1)
            vchain(vector, nc.vector.tensor_copy(m_s[:, :], pm[:, :]))
            nc.vector.scalar_tensor_tensor(
                msq[:, :],
                pm[:, :],
                1.0,
                m_s[:, :],
                op0=mybir.AluOpType.mult,
                op1=mybir.AluOpType.mult,
                accum_out=sq[:, :],
            ).then_inc(sem_sq, 1)
            vector.wait_ge(sem_s1, 1)
            vchain(
                vector,
                nc.vector.tensor_scalar(
                    den[:, :],
                    s1[:, :],
                    sq[:, :],
                    s1[:, :],
                    op0=mybir.AluOpType.mult,
                    op1=mybir.AluOpType.add,
                ),
            )
            vchain(vector, nc.vector.reciprocal(rcp[:, :], den[:, :]))
            nc.vector.tensor_scalar(
                v[:, :],
                m_s[:, :],
                sq[:, :],
                rcp[:, :],
                op0=mybir.AluOpType.mult,
                op1=mybir.AluOpType.mult,
            ).then_inc(sem_v, 1)
            # vdiag[i, b*16+d] = v[i, d] * (i == b); the sem_v wait also
            # orders this after the v write on the same engine
            vector.wait_ge(sem_v, 1)
            vector.wait_ge(sem_const, 4)
            nc.vector.tensor_mul(
                vdiag.ap().rearrange("i (b d) -> i b d", d=OUT_DIM),
                v[:, :].unsqueeze(1).broadcast_to([BPC, BPC, OUT_DIM]),
                dg01.ap().rearrange("i (b d) -> i b d", d=OUT_DIM),
            ).then_inc(sem_vd, 1)
            # pbc -> vb (3 copies) once the broadcast matmul is done
            vector.wait_ge(sem_bc, 1)
            nc.vector.tensor_copy(
                vb.ap().rearrange("p (t f) -> p t f", t=4),
                pbc[:, :].unsqueeze(1).broadcast_to([128, 4, BPC * OUT_DIM]),
            ).then_inc(sem_ob, 1)

        @block.tensor
        def _(tensor):
            for c in range(CK):
                if c % CQ == 0:
                    q = c // CQ
                    tensor.wait_ge(sem_xq[q], 16)
                    tensor.wait_ge(sem_wq[q], 16)
                mm = nc.tensor.matmul(
                    pm[:, :], xt_v[:, c, :], wf_v[:, c, :],
                    start=(c == 0), stop=(c == CK - 1),
                )
                if safe_sync and c < CK - 1:
                    mcount[0] += 1
                    mm.then_inc(msem, 1)
                    tensor.wait_ge(msem, mcount[0])
            mm.then_inc(sem_mm, 1)
            # pbc[p, b*16+d] = sum_i ones8[i, p] * vdiag[i, b*16+d] = v[b, d]
            tensor.wait_ge(sem_const, 4)  # ones8 + dg01 ready
            tensor.wait_ge(sem_vd, 1)
            nc.tensor.matmul(
                pbc[:, :], ones8.ap(), vdiag.ap(), start=True, stop=True
            ).then_inc(sem_bc, 1)

    return nc


def _host_prep(x, W, use_bf16=USE_BF16):
    Wf = np.asarray(W, np.float32)[0].reshape(K, OUT_DIM) * np.float32(1.0 / N)
    wf_host = np.ascontiguousarray(Wf.reshape(CK, 128, OUT_DIM).transpose(1, 0, 2))
    x = np.asarray(x, np.float32)
    if use_bf16:
        import ml_dtypes

        wf_host = wf_host.astype(ml_dtypes.bfloat16)
    in_maps = []
    for i in range(NCORES):
        xs = x[i * BPC : (i + 1) * BPC].reshape(BPC, CK, 128)
        xt_host = np.ascontiguousarray(xs.transpose(2, 1, 0))
        if use_bf16:
            import ml_dtypes

            xt_host = xt_host.astype(ml_dtypes.bfloat16)
        in_maps.append({"xt": xt_host, "wf": wf_host})
    return in_maps


def _unshard(results):
    out = np.empty((B, N, OUT_DIM), np.float32)
    for i in range(NCORES):
        o_np = results[i]["o"]  # [128, TJ, BPC, OUT_DIM] = (p, t, b, d)
        out[i * BPC : (i + 1) * BPC] = (
            o_np.transpose(2, 1, 0, 3).reshape(BPC, N, OUT_DIM)
        )
    return out


def kernel(x, W):
    global LAST_RESULT
    if "nc" not in _CACHE:
        _CACHE["nc"] = build_nc()
    nc = _CACHE["nc"]
    in_maps = _host_prep(x, W)
    trace = os.environ.get("KERNEL_TRACE") == "1"
    res = run_bass_kernel_spmd(nc, in_maps, list(range(NCORES)), trace=trace)
    LAST_RESULT = res
    return _unshard(res.results)

